# revision 1
# baseline (speedup 1.0000x reference)
"""Trainium2 Bass kernel for a FlowNet-style CorrelationLayer.

out[0, j*7+i, h, w] = sum_c x[0,c,h,w] * y[0,c,h+j-3,w+i-3]   (zero-padded y)

Shapes: x, y = [1, 128, 384, 512] fp32  ->  out = [1, 49, 384, 512] fp32.

Strategy
--------
* Shard H (rows) across the 8 NeuronCores: core k computes output rows
  [48k, 48k+48).  The y halo (3 rows each side) is sliced on the host from
  the full input, so no inter-core communication is needed.
* Per core, the C=128 contraction runs on the TensorEngine as "all-pairs"
  patch matmuls: lhsT = an 8x8 pixel patch of x (M=64 columns, K=C=128),
  rhs = the matching 14x14 halo patch of y (N=196 columns).  Entry
  (m=(a,b), n=(u,v)) of the PSUM block is the correlation of x pixel
  (a,b) with y pixel (u-3, v-3) relative to the patch origin; the useful
  entries are the 7x7 window u in [a, a+6], v in [b, b+6].  Two patches
  are packed into the 128 PE columns via tile_position col-tiling.
* PSUM blocks are evacuated (fp32 -> fp16) into a staging tile laid out
  st[m, n, t] with the row-band position t = (pr, q) innermost.  In that
  layout, for a fixed x-patch-row a, the whole useful u-window
  [a, a+6] x v x t is ONE contiguous run per partition, so the output
  DMA ships exactly the u-window (7 of 14 rows; amplification 2.0 vs the
  4.0 of shipping whole blocks) with large full-bandwidth descriptors and
  access patterns whose strides are exact partition steps (the BIR
  verifier rejects fractional partition steps, so the v-window which
  would need them stays on the host).  The band is split into three
  staging tiles (pr pairs) so earlier thirds ship while later thirds
  still compute.  Per-core HBM traffic: 6.3 (x) + 7.2 (y) + 4.8 (out)
  = 18.3 MB vs 22.6 MB for the whole-block scheme.
* The final v = b + i gather is a cheap numpy fancy-index on the host.
  Inputs are shipped as fp16 (quantization error ~4e-4 relative).
"""

import numpy as np

import bass_rust
import concourse.bass as bass  # noqa: F401  (AP types pulled in transitively)
import concourse.tile as tile
from concourse import bacc, mybir
from concourse.bass_utils import run_bass_kernel_spmd

B, C, H, W = 1, 128, 384, 512
NCORES = 8
HB = H // NCORES          # 48 output rows per core
PA, PB = 8, 8             # x patch: 8 rows x 8 cols = 64 = M per matmul
HA, HW_ = PA + 6, PB + 6  # y halo patch: 14 x 14
NF = HA * HW_             # 196 = N (matmul free size)
PR = HB // PA             # 6 patch-rows
PW = W // PB              # 64 patch-cols
PQ = PW // 2              # 32 pairs (two patches packed per 128 partitions)

STPRS = [(0, 2), (2, 4), (4, 5), (5, 6)]  # pr ranges per staging tile
TT01 = 2 * PQ             # 64: t positions in the pr-pair staging tiles
TT23 = PQ                 # 32: t positions in the single-pr staging tiles
RUN01 = 8 * HW_ * TT01    # 7168: run els per (a-pair, tile 0/1) DMA
RUN23 = 10 * HW_ * TT23   # 4480: run els per (a-quad, tile 2/3) DMA

F16 = mybir.dt.float16

_PROGRAM = None


def _build_program():
    nc = bacc.Bacc("TRN2", target_bir_lowering=False, debug=False)

    # x is pre-tiled on the host to [C, patch, m] so each patch's 64 weight
    # columns are contiguous (walrus requires a single free dim on the
    # stationary matmul operand).
    xb = nc.declare_dram_parameter("xb", [C, PR * PW, PA * PB], F16, isOutput=False)
    yb = nc.declare_dram_parameter("yb", [C, HB + 6, W + 6], F16, isOutput=False)
    # co01[k, ap, half, p, (u_rel, pr_local, q, v)]: p = 16 partitions
    # covering patch-rows {2ap, 2ap+1}, u_rel in [0, 8) = u - 2ap.
    co01 = nc.declare_dram_parameter("co01", [2, 4, 2, 16, RUN01], F16, isOutput=True)
    # co2[kk, aq, half, p, (u_rel, q, v)]: p = 32 partitions covering
    # patch-rows {4aq..4aq+3}, u_rel in [0, 10) = u - 4aq; kk = pr - 4.
    co2 = nc.declare_dram_parameter("co2", [2, 2, 2, 32, RUN23], F16, isOutput=True)

    with tile.TileContext(nc) as tc:
        with (
            tc.tile_pool(name="xpool", bufs=1) as xpool,
            tc.tile_pool(name="ypool", bufs=1) as ypool,
            tc.tile_pool(name="psum", bufs=4, space="PSUM") as psum_pool,
            tc.tile_pool(name="st", bufs=1) as st_pool,
        ):
            X = xpool.tile([C, PR * PW, PA * PB], F16)
            Y = ypool.tile([C, HB + 6, W + 6], F16)
            # st[m, u, t, v]: u-major so an a-group's u-window is one
            # contiguous run per partition, v innermost so the evacuation
            # copies keep a 14-element contiguous inner dim on both sides.
            ST = [
                st_pool.tile([128, HA, PQ * (p1 - p0), HW_], F16, name=f"st{k}")
                for k, (p0, p1) in enumerate(STPRS)
            ]

            # Issue input loads in the order the patch-row pipeline consumes
            # them (each HW queue drains FIFO): patch-row pr needs X chunk
            # pr//2 and Y rows [8pr, 8pr+14).  Y's first chunk is the full
    	    # 14 rows pr 0 needs.  Dispatch is split between the SP and Pool
            # queues with roughly balanced bytes (a dma_start costs
            # ~0.6-1us of dispatch time, and an engine's compute stalls
            # until its own queue drains, so ACT/DVE must stay clean).
            def load_x(pr0, pr1, eng):
                eng.dma_start(
                    X[:, pr0 * PW : pr1 * PW, :], xb[:, pr0 * PW : pr1 * PW, :]
                )

            def load_y(r0, r1, eng):
                eng.dma_start(Y[:, r0:r1, :], yb[:, r0:r1, :])

            load_y(0, 14, nc.sync)       # pr 0
            load_x(0, 1, nc.sync)        # pr 0
            load_y(14, 22, nc.sync)      # pr 1
            load_x(1, 2, nc.sync)        # pr 1
            load_y(22, 30, nc.sync)      # pr 2
            load_x(2, 3, nc.sync)        # pr 2
            load_y(30, 38, nc.sync)      # pr 3
            load_x(3, 4, nc.gpsimd)      # pr 3
            load_y(38, 46, nc.sync)      # pr 4
            load_x(4, 5, nc.sync)        # pr 4
            load_y(46, 54, nc.sync)      # pr 5
            load_x(5, 6, nc.gpsimd)      # pr 5

            for pr in range(PR):
                k = next(i for i, (p0, p1) in enumerate(STPRS) if p0 <= pr < p1)
                st = ST[k]
                for qq in range(0, PQ, 2):
                    # Four 8x8 patches (two col-tiled pairs) share one PSUM
                    # bank.
                    ps = psum_pool.tile([128, 2, 256], mybir.dt.float32)
                    for s in range(2):
                        q = qq + s
                        for half in range(2):
                            wp = 2 * q + half
                            lhsT = X[:, pr * PW + wp, :]
                            rhs = Y[
                                :, pr * PA : pr * PA + HA, wp * PB : wp * PB + HW_
                            ]
                            nc.tensor.matmul(
                                ps[half * 64 : (half + 1) * 64, s, :NF],
                                lhsT,
                                rhs,
                                start=True,
                                stop=True,
                                tile_position=(0, 64 * half),
                            )
                    # Evacuate to the staging layout st[m, u, t, v] with
                    # t = (pr - p0) * PQ + q.  Walk order (u, s, v) keeps a
                    # 14-element contiguous inner dim on both sides.
                    toff = (pr - STPRS[k][0]) * PQ + qq
                    dst = st[:, :, toff : toff + 2, :]
                    src = ps[:, :, :NF].rearrange("p s (u v) -> p u s v", u=HA)
                    # Alternate evacuation between DVE and ACT so neither
                    # becomes the bottleneck.
                    if (qq // 2) % 2 == 0:
                        nc.vector.tensor_copy(dst, src)
                    else:
                        nc.scalar.copy(dst, src)
                if pr == STPRS[k][1] - 1:
                    # This staging tile is complete: ship each patch-row
                    # group's shared u-window as one contiguous run over
                    # consecutive partitions (only dim0 of a DMA AP may cross
                    # partitions, and only with whole-partition strides; the
                    # host drops the extra u rows per pixel).  dma_starts
                    # cost ~1-1.5us of dispatch+semaphore time each, so keep
                    # them off ACT/DVE (evacuation copies) and use a-PAIRS
                    # (8 DMAs, 2.29x amplification) for the big early tiles
                    # but a-QUADS (4 DMAs, 2.86x) for the two single-pr tail
                    # tiles, whose shipping cannot overlap later compute.
                    tt = PQ * (STPRS[k][1] - STPRS[k][0])
                    stf = NF * tt
                    st_t = st[:, :, :].tensor
                    if k < 2:
                        for ap in range(4):
                            for half in range(2):
                                src = bass_rust.AP(
                                    st_t,
                                    (64 * half + 16 * ap) * stf
                                    + (2 * ap) * (HW_ * tt),
                                    [[stf, 16], [1, 8 * HW_ * tt]],
                                )
                                nc.gpsimd.dma_start(co01[k, ap, half], src)
                    else:
                        for aq in range(2):
                            for half in range(2):
                                src = bass_rust.AP(
                                    st_t,
                                    (64 * half + 32 * aq) * stf
                                    + (4 * aq) * (HW_ * tt),
                                    [[stf, 32], [1, 10 * HW_ * tt]],
                                )
                                eng = nc.sync if half else nc.gpsimd
                                eng.dma_start(co2[k - 2, aq, half], src)

    nc.compile()
    return nc


def _program():
    global _PROGRAM
    if _PROGRAM is None:
        _PROGRAM = _build_program()
    return _PROGRAM


def _make_in_maps(x: np.ndarray, y: np.ndarray):
    x0 = np.asarray(x[0]).astype(np.float16)
    # [C, H, W] -> [C, H/PA, PA, PW, PB] -> [C, H/PA, PW, PA, PB]
    xt = x0.reshape(C, H // PA, PA, PW, PB).transpose(0, 1, 3, 2, 4)
    xt = np.ascontiguousarray(xt.reshape(C, H // PA * PW, PA * PB))
    yp = np.zeros((C, H + 6, W + 6), np.float16)
    yp[:, 3 : 3 + H, 3 : 3 + W] = y[0]
    in_maps = []
    for k in range(NCORES):
        in_maps.append(
            {
                "xb": np.ascontiguousarray(xt[:, k * PR * PW : (k + 1) * PR * PW, :]),
                "yb": np.ascontiguousarray(yp[:, k * HB : k * HB + HB + 6, :]),
            }
        )
    return in_maps


_GATHER_IDX = None


def _gather_indices():
    global _GATHER_IDX
    if _GATHER_IDX is None:
        j = np.arange(7)[None, :]
        i = np.arange(7)[None, :]
        b = np.arange(PB)[:, None]
        ar2 = np.arange(2)[:, None]
        ar4 = np.arange(4)[:, None]
        _GATHER_IDX = (
            np.ascontiguousarray((ar2 + j).reshape(1, 1, 1, 2, 1, 7, 1, 1, 1)),
            np.ascontiguousarray((b + i).reshape(1, 1, 1, 1, PB, 1, 1, 1, 7)),
            np.ascontiguousarray((ar4 + j).reshape(1, 1, 1, 4, 1, 7, 1, 1)),
            np.ascontiguousarray((b + i).reshape(1, 1, 1, 1, PB, 1, 1, 7)),
        )
    return _GATHER_IDX


def _gather_core(co01_k: np.ndarray, co2_k: np.ndarray) -> np.ndarray:
    """Device outputs -> [49, HB, W] band of the output."""
    iu2, iv2, iu4, iv4 = _gather_indices()
    a = co01_k.reshape(2, 4, 2, 2, PB, 8, 2, PQ, HW_)
    # [k, ap, half, ar, b, urel, prl, q, v]: urel = ar + j, then v = b + i
    g = np.take_along_axis(a, iu2, axis=5)
    g = np.take_along_axis(g, iv2, axis=8)
    # -> [j, i, k, prl, ap, ar, q, half, b] = rows 0..31
    top = g.transpose(5, 8, 0, 6, 1, 3, 7, 2, 4).reshape(49, 32, W)
    a = co2_k.reshape(2, 2, 2, 4, PB, 10, PQ, HW_)
    # [kk, aq, half, ar, b, urel, q, v]
    g = np.take_along_axis(a, iu4, axis=5)
    g = np.take_along_axis(g, iv4, axis=7)
    # -> [j, i, kk, aq, ar, q, half, b] = rows 32..47
    bot = g.transpose(5, 7, 0, 1, 3, 6, 2, 4).reshape(49, 16, W)
    return np.concatenate([top, bot], axis=1)


def _run(in_maps, trace=False, **kw):
    return run_bass_kernel_spmd(
        _program(), in_maps, core_ids=list(range(NCORES)), trace=trace, **kw
    )


def kernel(x: np.ndarray, y: np.ndarray) -> np.ndarray:
    x = np.asarray(x)
    y = np.asarray(y)
    res = _run(_make_in_maps(x, y)).results
    out = np.empty((1, 49, H, W), np.float32)
    for k in range(NCORES):
        out[0, :, k * HB : (k + 1) * HB, :] = _gather_core(
            np.asarray(res[k]["co01"]), np.asarray(res[k]["co2"])
        ).astype(np.float32)
    return out



# revision 4
# speedup vs baseline: 1.0222x; 1.0222x over previous
"""Trainium2 Bass kernel for a FlowNet-style CorrelationLayer.

out[0, j*7+i, h, w] = sum_c x[0,c,h,w] * y[0,c,h+j-3, w+i-3]   (zero-padded y)

Shapes: x, y = [1, 128, 384, 512] fp32  ->  out = [1, 49, 384, 512] fp32.

Strategy (v2)
-------------
* Shard H (rows) across the 8 NeuronCores: core k computes output rows
  [48k, 48k+48).  The y halo (3 rows each side) is sliced on the host, so no
  inter-core communication is needed.
* y ships as fp8 e3m4 (half the bytes of fp16) and feeds the TensorEngine
  rhs DIRECTLY: the PE upconverts operands to ~fp22 internally, so a mixed
  fp16(x-weights) x fp8e3(y) matmul is exact in the quantized values.  The
  e3m4 quantization of one operand costs ~1.34e-2 relative error (measured),
  well under the 2e-2 gate.  Quantizing BOTH operands (1.9e-2) is too risky,
  so x stays fp16.
* Patches are 16 rows x 4 cols (M=64, two patches col-tiled per PSUM tile).
  The y halo patch is 22 x 10, N=220.  Narrow (PB=4) patches shrink the
  shipped v-extent to 10, cutting output amplification to 8*10/49 = 1.63
  (vs 2.29 at 8x8): out = 3.94 MB/core vs 5.96.
* Staging st[m, u, t, v] per patch-row pr; the output DMA ships, per a-PAIR
  of x rows, the shared u-window [2ap, 2ap+8) as one big contiguous run per
  partition (the BIR verifier only allows whole-partition dim0 strides, so
  the exact per-pixel windows are finished by a cheap host-side gather).
  The last patch-row ships in two t-halves so the final transfer is small.
* Input chunks are spread over four dispatch queues (sync/gpsimd/tensor/
  scalar engines) sized so each chunk lands just before the matmuls that
  need it; outputs queue behind inputs on sync/gpsimd (the stream is
  input-bound until ~40us anyway) and the final half-tile fans out across
  four engines to minimize dispatch-serialized tail.
* Per-core HBM traffic: 6.3 (x fp16) + 3.6 (y fp8) + 3.9 (out) = 13.8 MB
  vs 19.4 MB for the v1 kernel.
"""

import numpy as np
import ml_dtypes

import bass_rust
import concourse.bass as bass  # noqa: F401  (AP types pulled in transitively)
import concourse.tile as tile
from concourse import bacc, mybir
from concourse.bass_utils import run_bass_kernel_spmd

B, C, H, W = 1, 128, 384, 512
NCORES = 8
HB = H // NCORES          # 48 output rows per core
PA, PB = 16, 4            # x patch: 16 rows x 4 cols = 64 = M per matmul
HA, HV = PA + 6, PB + 6   # y halo patch: 22 x 10
NF = HA * HV              # 220 = N (matmul free size)
PR = HB // PA             # 3 patch-rows
PW = W // PB              # 128 patch-cols
NQ = PW // 2              # 64 col-tile pairs (two patches per 128 partitions)
STF = HA * NQ * HV        # 14080 staging elems per partition per tile
RUNF = 8 * NQ * HV        # 5120: run elems per (a-pair) full-tile DMA
RUNH = HV * (NQ // 2)     # 320: run elems per u-row in half-tile DMAs

F16 = mybir.dt.float16
F8 = mybir.dt.float8e3
E3M4 = ml_dtypes.float8_e3m4

_PROGRAM = None


def _build_program():
    nc = bacc.Bacc("TRN2", target_bir_lowering=False, debug=False)

    # x pre-tiled on the host to [C, patch, m] (m = a*4 + b, a-major) so each
    # patch's 64 weight columns are contiguous.
    xb = nc.declare_dram_parameter("xb", [C, PR * PW, PA * PB], F16, isOutput=False)
    yb = nc.declare_dram_parameter("yb", [C, HB + 6, W + 6], F8, isOutput=False)
    # coa[pr, ap, half, p, (u_rel, q, v)]: p = 8 partitions covering
    # a in {2ap, 2ap+1} x b, u_rel = u - 2ap in [0, 8).
    coa = nc.declare_dram_parameter("coa", [2, 8, 2, 8, RUNF], F16, isOutput=True)
    # cob[kk, ap, half, p, u_rel, (tr, v)]: pr 2 shipped in two t-halves kk.
    cob = nc.declare_dram_parameter("cob", [2, 8, 2, 8, 8, RUNH], F16, isOutput=True)

    with tile.TileContext(nc) as tc:
        with (
            tc.tile_pool(name="xpool", bufs=1) as xpool,
            tc.tile_pool(name="ypool", bufs=1) as ypool,
            tc.tile_pool(name="psum", bufs=4, space="PSUM") as psum_pool,
            tc.tile_pool(name="st", bufs=1) as st_pool,
        ):
            X = xpool.tile([C, PR * PW, PA * PB], F16)
            Y = ypool.tile([C, HB + 6, W + 6], F8)
            # st[m, u, t, v]: u-major so an a-pair's u-window is one
            # contiguous run per partition; (t, v) innermost so evacuation
            # writes land contiguously per (u, s) step.
            ST = [
                st_pool.tile([128, HA, NQ, HV], F16, name=f"st{k}") for k in range(PR)
            ]

            # Input loads, spread over four dispatch queues (one per engine)
            # and ordered so each queue's cumulative bytes arrive just before
            # the matmuls that need them.  y0 (22 rows) is the long pole for
            # the first matmul, so it is split three ways; x patches stream
            # in quarter/half-pr chunks.
            def ly(r0, r1, eng):
                eng.dma_start(Y[:, r0:r1, :], yb[:, r0:r1, :])

            def lx(p0, p1, eng):
                eng.dma_start(X[:, p0:p1, :], xb[:, p0:p1, :])

            # Only gpsimd/sync/scalar can issue DMAs.  scalar fires three
            # early loads (before its first evacuation), then sync/gpsimd
            # carry the rest in need order.
            lx(0, 16, nc.scalar)
            ly(0, 8, nc.sync)
            ly(8, 15, nc.gpsimd)
            ly(15, 22, nc.scalar)
            lx(16, 32, nc.sync)
            lx(32, 48, nc.gpsimd)
            lx(48, 64, nc.scalar)
            lx(64, 96, nc.sync)
            lx(96, 128, nc.gpsimd)
            ly(22, 30, nc.sync)       # y for pr 1
            ly(30, 38, nc.gpsimd)
            lx(128, 160, nc.sync)
            lx(160, 192, nc.gpsimd)
            lx(192, 224, nc.sync)
            lx(224, 256, nc.gpsimd)
            ly(38, 46, nc.sync)       # y for pr 2
            ly(46, 54, nc.gpsimd)
            lx(256, 288, nc.sync)
            lx(288, 320, nc.gpsimd)
            lx(320, 352, nc.sync)
            lx(352, 384, nc.gpsimd)

            def ship_full(pr, eng_of):
                # 16 descriptors: whole-partition dim0 strides only, so one
                # descriptor per (a-pair, half) ships u in [2ap, 2ap+8) x
                # all t x all v as a single 10 KB run per partition.
                st_t = ST[pr][:, :, :].tensor
                for ap in range(8):
                    for half in range(2):
                        src = bass_rust.AP(
                            st_t,
                            (64 * half + 8 * ap) * STF + (2 * ap) * (NQ * HV),
                            [[STF, 8], [1, RUNF]],
                        )
                        eng_of(ap, half).dma_start(coa[pr, ap, half], src)

            def ship_half(kk, t0, eng_of):
                st_t = ST[2][:, :, :].tensor
                for ap in range(8):
                    for half in range(2):
                        src = bass_rust.AP(
                            st_t,
                            (64 * half + 8 * ap) * STF
                            + (2 * ap) * (NQ * HV)
                            + t0 * HV,
                            [[STF, 8], [NQ * HV, 8], [1, RUNH]],
                        )
                        eng_of(ap, half).dma_start(cob[kk, ap, half], src)

            for pr in range(PR):
                st = ST[pr]
                for qq in range(0, NQ, 2):
                    # Four 16x4 patches (two col-tiled pairs) share one PSUM
                    # bank: [128, 2, 220] fp32 = 1760 B of the 2 KB bank.
                    ps = psum_pool.tile([128, 2, NF], mybir.dt.float32)
                    for s in range(2):
                        q = qq + s
                        for half in range(2):
                            wp = 2 * q + half
                            lhsT = X[:, pr * PW + wp, :]
                            rhs = Y[
                                :, pr * PA : pr * PA + HA, wp * PB : wp * PB + HV
                            ]
                            nc.tensor.matmul(
                                ps[half * 64 : (half + 1) * 64, s, :NF],
                                lhsT,
                                rhs,
                                start=True,
                                stop=True,
                                tile_position=(0, 64 * half),
                            )
                    # Evacuate (fp32 -> fp16) into st[m, u, t, v]; src walked
                    # (s, u, v) so the PSUM read is one contiguous 440-elem
                    # run per partition.  Alternate DVE / ACT.
                    dst = st[:, :, qq : qq + 2, :].rearrange("p u s v -> p s u v")
                    src = ps[:, :, :NF].rearrange("p s (u v) -> p s u v", u=HA)
                    if (qq // 2) % 2 == 0:
                        nc.vector.tensor_copy(dst, src)
                    else:
                        nc.scalar.copy(dst, src)
                    if pr == 2 and qq == NQ // 2 - 2:
                        # First t-half of the last patch-row is complete:
                        # ship it while the second half computes.
                        ship_half(
                            0,
                            0,
                            lambda ap, half: nc.gpsimd if half else nc.sync,
                        )
                if pr < 2:
                    ship_full(
                        pr, lambda ap, half: nc.gpsimd if half else nc.sync
                    )
            # Tail: fan the final 16 descriptors across the three DMA-capable
            # engines so the dispatch-serialized tail is ~6 x 0.6us not 16x.
            tail_engines = [nc.gpsimd, nc.sync, nc.scalar]
            ship_half(1, NQ // 2, lambda ap, half: tail_engines[(2 * ap + half) % 3])

    nc.compile()
    return nc


def _program():
    global _PROGRAM
    if _PROGRAM is None:
        _PROGRAM = _build_program()
    return _PROGRAM


def _make_in_maps(x: np.ndarray, y: np.ndarray):
    x0 = np.asarray(x[0]).astype(np.float16)
    # [C, H, W] -> [C, H/PA, PA, PW, PB] -> [C, H/PA, PW, PA, PB]
    xt = x0.reshape(C, H // PA, PA, PW, PB).transpose(0, 1, 3, 2, 4)
    xt = np.ascontiguousarray(xt.reshape(C, H // PA * PW, PA * PB))
    yp = np.zeros((C, H + 6, W + 6), E3M4)
    yp[:, 3 : 3 + H, 3 : 3 + W] = np.asarray(y[0]).astype(E3M4)
    in_maps = []
    for k in range(NCORES):
        in_maps.append(
            {
                "xb": np.ascontiguousarray(xt[:, k * PR * PW : (k + 1) * PR * PW, :]),
                "yb": np.ascontiguousarray(yp[:, k * HB : k * HB + HB + 6, :]),
            }
        )
    return in_maps


_GATHER_IDX = None


def _gather_indices():
    global _GATHER_IDX
    if _GATHER_IDX is None:
        j = np.arange(7)[None, :]
        i = np.arange(7)[None, :]
        ar = np.arange(2)[:, None]
        b = np.arange(PB)[:, None]
        _GATHER_IDX = (
            np.ascontiguousarray((ar + j).reshape(1, 1, 1, 2, 1, 7, 1, 1)),
            np.ascontiguousarray((b + i).reshape(1, 1, 1, 1, PB, 1, 1, 7)),
        )
    return _GATHER_IDX


def _gather_core(coa_k: np.ndarray, cob_k: np.ndarray) -> np.ndarray:
    """Device outputs -> [49, HB, W] band of the output."""
    iu, iv = _gather_indices()
    # [pr, ap, half, ar, b, urel, q, v]: urel = ar + j, then v = b + i
    a = coa_k.reshape(2, 8, 2, 2, PB, 8, NQ, HV)
    g = np.take_along_axis(a, iu, axis=5)
    g = np.take_along_axis(g, iv, axis=7)
    # -> [j, i, pr, ap, ar, q, half, b] = rows 0..31
    top = g.transpose(5, 7, 0, 1, 3, 6, 2, 4).reshape(49, 32, W)
    # [kk, ap, half, ar, b, urel, tr, v]
    bb = cob_k.reshape(2, 8, 2, 2, PB, 8, NQ // 2, HV)
    g = np.take_along_axis(bb, iu, axis=5)
    g = np.take_along_axis(g, iv, axis=7)
    # -> [j, i, ap, ar, kk, tr, half, b] = rows 32..47
    bot = g.transpose(5, 7, 1, 3, 0, 6, 2, 4).reshape(49, 16, W)
    return np.concatenate([top, bot], axis=1)


def _run(in_maps, trace=False, **kw):
    return run_bass_kernel_spmd(
        _program(), in_maps, core_ids=list(range(NCORES)), trace=trace, **kw
    )


def kernel(x: np.ndarray, y: np.ndarray) -> np.ndarray:
    x = np.asarray(x)
    y = np.asarray(y)
    res = _run(_make_in_maps(x, y)).results
    out = np.empty((1, 49, H, W), np.float32)
    for k in range(NCORES):
        out[0, :, k * HB : (k + 1) * HB, :] = _gather_core(
            np.asarray(res[k]["coa"]), np.asarray(res[k]["cob"])
        ).astype(np.float32)
    return out
